# revision 1
# baseline (speedup 1.0000x reference)
"""Trainium2 Bass kernel for nn_Coupling: out[e, s*J+j] = sum_a feat[e, a*S+s] * P[a, j].

Sharding: env axis data-parallel across 8 cores (3750 envs/core); P is tiny and
built host-side, replicated to every core as a [108, 30] block-diagonal input.

Per-core device kernel:
  - K-packs 3 envs into one matmul contraction (K = 3*36 = 108 partitions),
    with features as the stationary operand and the 3-env block-diagonal P as
    the 30-column moving operand.
  - For each env-triple, G matmuls: matmul r uses feature columns s = r (mod G),
    so output partition m holds s = G*m + r.  Per partition the (s_lo, j) block
    is G*10 floats contiguous and 64B-aligned in DRAM, keeping the output DMA
    efficient despite the s-major/j-minor interleaved output layout.
  - 128/(S/G) triples share one PSUM batch (col-tiling); one DVE copy per batch
    permutes (r,t,j) -> (t,r,j) into an SBUF staging buffer; one large DMA per
    partition-quarter flushes each stage.
"""

import math

import numpy as np

import concourse.mybir as mybir
from concourse import bacc, tile
from concourse.bass_utils import run_bass_kernel_spmd

ENV = 30000
A = 36          # n_alpha
S = 256         # soap
J = 10          # n_j
N_CORES = 8
E_CORE = ENV // N_CORES  # 3750

T = 3           # envs packed into one matmul contraction (K = T*A = 108)
G = 8           # s values interleaved per output partition (run = G*J*4 bytes)
NT_LD = 4       # triples per feature-load DMA
NB = 8          # PSUM batches per output stage

F32 = mybir.dt.float32

_NC_CACHE = {}


def build_nc(n_env, g=G, fbufs=12, stbufs=2, psbufs=8, dma_only=False):
    assert n_env % T == 0
    n_tri = n_env // T
    part = S // g           # output partitions per triple
    nt_ps = 128 // part     # triples per PSUM batch
    st_tri = NB * nt_ps     # triples per stage

    nc = bacc.Bacc("TRN2", target_bir_lowering=False, debug=False)

    feat = nc.dram_tensor("features", [n_env, A * S], F32, kind="ExternalInput")
    pblk = nc.dram_tensor("pblk", [T * A, T * J], F32, kind="ExternalInput")
    out = nc.dram_tensor("out", [n_env, S * J], F32, kind="ExternalOutput")

    feat3 = feat.rearrange("e (a s) -> e a s", a=A)
    out3 = out.rearrange("e (sh x) -> e sh x", sh=part)  # x = s_lo*J + j

    with tile.TileContext(nc) as tc:
        with (
            tc.tile_pool(name="const", bufs=1) as cpool,
            tc.tile_pool(name="feat", bufs=fbufs) as fpool,
            tc.tile_pool(name="psum", bufs=psbufs, space="PSUM") as pspool,
            tc.tile_pool(name="stage", bufs=stbufs) as stpool,
        ):
            pb = cpool.tile([T * A, T * J], F32)
            nc.sync.dma_start(pb[:], pblk[:])
            dummy = None
            if dma_only:
                dummy = cpool.tile([128, NB, T, g * J], F32)
                nc.gpsimd.memset(dummy[:], 0.0)

            tri0 = 0
            while tri0 < n_tri:
                n_tri_st = min(st_tri, n_tri - tri0)
                n_grp = math.ceil(n_tri_st / NT_LD)
                e0 = tri0 * T

                # load feature groups (NT_LD consecutive triples each)
                fts = []
                for gi in range(n_grp):
                    nt = min(NT_LD, n_tri_st - gi * NT_LD)
                    eg = e0 + gi * NT_LD * T
                    ft = fpool.tile([T * A, nt, S], F32)
                    nc.sync.dma_start(
                        ft[:],
                        feat3[eg : eg + nt * T].rearrange(
                            "(m t) a s -> t a m s", t=T
                        ),
                    )
                    fts.append(ft.rearrange("p m (sh g) -> p m g sh", g=g))

                if not dma_only:
                    stage = stpool.tile([128, NB, T, g * J], F32)
                    # triple tau -> quarter q = tau//NB, psum batch b = tau%NB
                    for b in range(min(NB, n_tri_st)):
                        nq = sum(1 for q in range(nt_ps) if NB * q + b < n_tri_st)
                        ps = pspool.tile([128, g, T, J], F32)
                        for q in range(nq):
                            tau = NB * q + b
                            gi, mm = divmod(tau, NT_LD)
                            for r in range(g):
                                nc.tensor.matmul(
                                    ps[q * part : (q + 1) * part, r],
                                    fts[gi][:, mm, r],
                                    pb[:],
                                    tile_position=(0, q * part),
                                )
                        nc.vector.tensor_copy(
                            stage[: nq * part, b].rearrange(
                                "p t (r j) -> p t r j", r=g
                            ),
                            ps[: nq * part].rearrange("p r t j -> p t r j"),
                        )
                else:
                    stage = dummy

                # flush stage: quarter q covers triples [NB*q, NB*q+NB)
                for q in range(math.ceil(n_tri_st / NB)):
                    nb_q = min(NB, n_tri_st - NB * q)
                    eq = e0 + NB * q * T
                    nc.scalar.dma_start(
                        out3[eq : eq + nb_q * T].rearrange("e sh x -> sh e x"),
                        stage[q * part : (q + 1) * part, :nb_q],
                    )

                tri0 += n_tri_st

    nc.compile()
    return nc


def _get_nc(n_env, **kw):
    key = (n_env, tuple(sorted(kw.items())))
    if key not in _NC_CACHE:
        _NC_CACHE[key] = build_nc(n_env, **kw)
    return _NC_CACHE[key]


def make_pblk(U, alpha1, alpha2, j1, j2):
    P = (U[alpha1][:, j1] * U[alpha2][:, j2]).astype(np.float32)  # [A, J]
    pblk = np.zeros((T * A, T * J), dtype=np.float32)
    for t in range(T):
        pblk[t * A : (t + 1) * A, t * J : (t + 1) * J] = P
    return pblk


def run_spmd(features, U, alpha1, alpha2, j1, j2, trace=False, **kw):
    features = np.asarray(features, dtype=np.float32)
    pblk = make_pblk(
        np.asarray(U), np.asarray(alpha1), np.asarray(alpha2),
        np.asarray(j1), np.asarray(j2),
    )
    nc = _get_nc(E_CORE, **kw)
    in_maps = [
        {"features": features[c * E_CORE : (c + 1) * E_CORE], "pblk": pblk}
        for c in range(N_CORES)
    ]
    res = run_bass_kernel_spmd(nc, in_maps, list(range(N_CORES)), trace=trace)
    out = np.concatenate([res.results[c]["out"] for c in range(N_CORES)], axis=0)
    return out, res


def kernel(features, U, alpha1, alpha2, j1, j2):
    return run_spmd(features, U, alpha1, alpha2, j1, j2)[0]



# revision 6
# speedup vs baseline: 2.3514x; 2.3514x over previous
"""Trainium2 Bass kernel for nn_Coupling: out[e, s*J+j] = sum_a feat[e, a*S+s] * P[a, j].

Sharding: env axis data-parallel across 8 cores (3750 envs/core); P is tiny and
built host-side, replicated to every core as a [108, 30] block-diagonal input.

bf16 pipeline (error budget ~3e-3 « 2e-2 gate): host casts features/P to bf16,
device computes bf16 matmuls with fp32 PSUM accumulation, writes bf16 output,
host casts back to fp32.  Halves HBM traffic on both sides.

Per-core device kernel:
  - K-packs 3 envs into one contraction (K = 3*36 = 108 partitions) against the
    block-diagonal P ([108, 30] moving operand, 30 = 3 envs x 10 j).
  - A batch = up to 128 triples (384 envs).  Phase r (r = 0..255) does ONE
    matmul: stationary ft[:, :, r] ([108, n_tri_b] -> output partition = triple),
    moving P-block, output [n_tri_b, 30] into PSUM at phase slot r.  Every
    feature column crosses LDWEIGHTS exactly once (the PE-side floor).
  - PSUM phase slots are padded to 32 elements (128 B) so a 30-float matmul
    output never straddles a 2 KB PSUM bank; 64 phases per PSUM tile (4 banks).
  - One DVE copy per PSUM tile permutes (r,t,j) -> (t,r,j) bf16 into a stage
    buffer whose per-partition rows are that triple's 3 full env rows --
    so the flush is a single fully-contiguous ~2 MB DMA per batch.
"""

import numpy as np
import ml_dtypes

import concourse.mybir as mybir
from concourse import bacc, tile
from concourse.bass_utils import run_bass_kernel_spmd

ENV = 30000
A = 36          # n_alpha
S = 256         # soap
J = 10          # n_j
N_CORES = 8
E_CORE = ENV // N_CORES  # 3750

T = 3           # envs packed into one contraction (K = T*A = 108)
K = T * A
TRI_B = 128     # triples per batch (= output partitions per batch)
RB = 64         # phases per PSUM tile (64 * 32 * 4 B = 4 banks)

F32 = mybir.dt.float32
BF16 = mybir.dt.bfloat16

_NC_CACHE = {}


def build_nc(n_env, tri_b=TRI_B, fbufs=2, stbufs=2, psbufs=2):
    assert n_env % T == 0
    n_tri = n_env // T

    nc = bacc.Bacc("TRN2", target_bir_lowering=False, debug=False)

    feat = nc.dram_tensor("features", [n_env, A * S], BF16, kind="ExternalInput")
    pblk = nc.dram_tensor("pblk", [K, T * J], BF16, kind="ExternalInput")
    out = nc.dram_tensor("out", [n_env, S * J], BF16, kind="ExternalOutput")

    feat3 = feat.rearrange("e (a s) -> e a s", a=A)

    with tile.TileContext(nc) as tc:
        with (
            tc.tile_pool(name="const", bufs=1) as cpool,
            tc.tile_pool(name="feat", bufs=fbufs) as fpool,
            tc.tile_pool(name="psum", bufs=psbufs, space="PSUM") as pspool,
            tc.tile_pool(name="stage", bufs=stbufs) as stpool,
        ):
            pb = cpool.tile([K, T * J], BF16)
            nc.sync.dma_start(pb[:], pblk[:])

            for tb in range(0, n_tri, tri_b):
                ntb = min(tri_b, n_tri - tb)
                eb = tb * T

                ft = fpool.tile([K, ntb, S], BF16)
                nc.sync.dma_start(
                    ft[:],
                    feat3[eb : eb + ntb * T].rearrange(
                        "(m t) a s -> t a m s", t=T
                    ),
                )

                stage = stpool.tile([ntb, T, S * J], BF16)
                stage4 = stage.rearrange("p t (r j) -> p t r j", j=J)

                for blk in range(S // RB):
                    ps = pspool.tile([ntb, RB, 32], F32)
                    for rr in range(RB):
                        r = blk * RB + rr
                        nc.tensor.matmul(
                            ps[:, rr, 0:J * T], ft[:, :, r], pb[:]
                        )
                    nc.vector.tensor_copy(
                        stage4[:, :, blk * RB : (blk + 1) * RB],
                        ps[:, :, 0:J * T].rearrange("p r (t j) -> p t r j", t=T),
                    )

                nc.scalar.dma_start(
                    out[eb : eb + ntb * T].rearrange("(m t) x -> m t x", t=T),
                    stage[:],
                )

    nc.compile()
    return nc


def _get_nc(n_env, **kw):
    key = (n_env, tuple(sorted(kw.items())))
    if key not in _NC_CACHE:
        _NC_CACHE[key] = build_nc(n_env, **kw)
    return _NC_CACHE[key]


def make_pblk(U, alpha1, alpha2, j1, j2):
    P = (U[alpha1][:, j1] * U[alpha2][:, j2]).astype(np.float32)  # [A, J]
    pblk = np.zeros((K, T * J), dtype=np.float32)
    for t in range(T):
        pblk[t * A : (t + 1) * A, t * J : (t + 1) * J] = P
    return pblk.astype(ml_dtypes.bfloat16)


def run_spmd(features, U, alpha1, alpha2, j1, j2, trace=False, **kw):
    features = np.asarray(features, dtype=np.float32).astype(ml_dtypes.bfloat16)
    pblk = make_pblk(
        np.asarray(U), np.asarray(alpha1), np.asarray(alpha2),
        np.asarray(j1), np.asarray(j2),
    )
    nc = _get_nc(E_CORE, **kw)
    in_maps = [
        {"features": features[c * E_CORE : (c + 1) * E_CORE], "pblk": pblk}
        for c in range(N_CORES)
    ]
    res = run_bass_kernel_spmd(nc, in_maps, list(range(N_CORES)), trace=trace)
    out = np.concatenate(
        [res.results[c]["out"] for c in range(N_CORES)], axis=0
    ).astype(np.float32)
    return out, res


def kernel(features, U, alpha1, alpha2, j1, j2):
    return run_spmd(features, U, alpha1, alpha2, j1, j2)[0]
